# revision 16
# baseline (speedup 1.0000x reference)
"""Multi-head self-attention Trainium2 kernel (8-core SPMD).

Sharding: core c handles (batch b = c // 2, head-group g = c % 2); the two
head-group partials per batch are summed on the host (tensor-parallel
unshard, as in the sharding hint's "all-reduce after out_proj").

Host-side shard prep folds the two projection matrices per stream
(W_qp @ W_q etc.) into one combined matrix — pure reassociation of the
linear chain — and pre-rounds all operands to the bf16 compute dtype.

Per-core device program (feature-major activations):
  qT = Wc_q^T.T-contraction of x^T   [512, 2048]   (same for kT)
  v  = token-major projection with a ones column per head (v_aug)
  per head pair, per 512-query chunk:
    S^T tile [128 keys, 2 x 512] via row-packed K=64 matmul pairs
    P^T = exp(S^T / 8) on ScalarE (PSUM -> SBUF bf16)
    O^T[65, 512] += v_aug_h.T @ P^T  (row 64 accumulates softmax denoms)
    normalize via reciprocal + ones-outer-product broadcast, into yT bf16
  out = yT.T-contraction with W_out^T slice, accumulated over pairs.
"""

import numpy as np
from contextlib import ExitStack

import concourse.bass as bass
import concourse.tile as tile
from concourse import mybir, bacc
from concourse.bass import ts, ds

FP = mybir.dt.float32r
BF = mybir.dt.bfloat16
F32 = mybir.dt.float32
EXP = mybir.ActivationFunctionType.Exp

N_CORES = 8
E = 1024          # embed dim
R = 512           # per-core head-group width (8 heads x 64)
L = 2048          # sequence length
D = 64            # head dim
ET = E // 128     # 8 embed tiles
RT = R // 128     # 4 r tiles == head pairs
LT = L // 128     # 16 token tiles
NJ = L // 128     # 16 key tiles
ICS = 512         # query chunk per PV accumulation
NIC = L // ICS    # 4
VW = 65           # v_aug width per head (64 + ones)


def emit(tc, outs, ins):
    with ExitStack() as ctx:
        _emit_body(tc, ctx, outs, ins)


def _emit_body(tc, ctx, outs, ins):
    nc = tc.nc

    x_d = ins["xT"]          # [1024, 2048] bf16  (x[b].T)
    wc_d = {"wq": ins["wcqT"], "wk": ins["wckT"], "wv": ins["wcvT"]}  # [1024, 512] bf16
    woutT_d = ins["woutT"]   # [512, 1024] bf16
    out_d = outs["out"]      # [2048, 1024] f32

    # ---------------- pools ----------------
    persist = ctx.enter_context(tc.tile_pool(name="persist", bufs=1))  # long-lived sbuf
    ptp = ctx.enter_context(tc.tile_pool(name="ptp", bufs=6))          # exp outputs
    dsb = ctx.enter_context(tc.tile_pool(name="dsb", bufs=2))          # denom broadcast
    outp = ctx.enter_context(tc.tile_pool(name="outp", bufs=2))        # final out staging
    ps_mm = ctx.enter_context(tc.tile_pool(name="ps_mm", space="PSUM", bufs=2))   # [128,512] proj/outproj
    ps_st = ctx.enter_context(tc.tile_pool(name="ps_st", space="PSUM", bufs=2))   # [128,1024] scores
    ps_pv = ctx.enter_context(tc.tile_pool(name="ps_pv", space="PSUM", bufs=2))   # [65,512] pv accum

    # ---------------- warm the Exp table off the critical path ----
    warm = persist.tile([1, 8], BF, name="warm", tag="warm")
    nc.vector.memset(warm[:], 0.0)
    warm2 = persist.tile([1, 8], BF, name="warm2", tag="warm2")
    nc.scalar.activation(warm2[:], warm[:], EXP, scale=0.125)

    # ---------------- load combined weights + x^T (DMA order: qk first) -------
    def load_wc(wname):
        w3 = wc_d[wname].rearrange("(t p) r -> t p r", p=128)  # [8,128,512]
        tiles = []
        for e in range(ET):
            wb = persist.tile([128, R], BF, name=f"{wname}c{e}", tag=f"{wname}c{e}")
            nc.sync.dma_start(wb[:], w3[e])
            tiles.append(wb)
        return tiles

    x3 = x_d.rearrange("(t p) l -> t p l", p=128)  # [8,128,2048]
    xT = [persist.tile([128, L], BF, name=f"xT{t}", tag=f"xT{t}") for t in range(ET)]

    def load_x_chunk(lc):
        for t in range(ET):
            nc.sync.dma_start(xT[t][:, ts(lc, 512)], x3[t, :, ts(lc, 512)])

    wc = {}
    wc["wq"] = load_wc("wq")
    wc["wk"] = load_wc("wk")
    load_x_chunk(0)
    wc["wv"] = load_wc("wv")
    for lc in range(1, 4):
        load_x_chunk(lc)

    wo3 = woutT_d.rearrange("(t p) e -> t p e", p=128)  # [4,128,1024]
    woutT = []
    for t in range(RT):
        wb = persist.tile([128, E], BF, name=f"woutT{t}", tag=f"woutT{t}")
        nc.sync.dma_start(wb[:], wo3[t])
        woutT.append(wb)

    # ---------------- v projection (token-major, with ones cols) ----------------
    v_aug = [persist.tile([128, 8 * VW], BF, name=f"vaug{lt}", tag=f"vaug{lt}")
             for lt in range(LT)]

    def emit_v_proj(lts):
        for lt in lts:
            va = v_aug[lt]
            va3 = va.rearrange("p (h w) -> p h w", w=VW)
            nc.vector.memset(va[:], 1.0)
            ps = ps_mm.tile([128, R], F32, name=f"v_ps{lt}", tag="ps_mm")
            for e in range(ET):
                nc.tensor.matmul(ps[:], xT[e][:, ts(lt, 128)], wc["wv"][e][:],
                                 start=(e == 0), stop=(e == ET - 1))
            ps3 = ps.rearrange("p (h d) -> p h d", d=D)
            nc.vector.tensor_copy(va3[:, :, 0:D], ps3[:])

    # ---------------- q/k projections for one pair ----------------
    qT = [None] * RT
    kT = [None] * RT

    def emit_qk_proj(p):
        for which, store in (("wq", qT), ("wk", kT)):
            dst = persist.tile([128, L], BF, name=f"{which}T{p}", tag=f"{which}T{p}")
            for lc in range(4):
                ps = ps_mm.tile([128, 512], F32, name=f"qk_ps{p}_{which}_{lc}", tag="ps_mm")
                for e in range(ET):
                    nc.tensor.matmul(ps[:], wc[which][e][:, ts(p, 128)],
                                     xT[e][:, ts(lc, 512)],
                                     start=(e == 0), stop=(e == ET - 1))
                nc.vector.tensor_copy(dst[:, ts(lc, 512)], ps[:])
            store[p] = dst

    # ---------------- attention (one pair, one query chunk) ----------------
    yT = [persist.tile([128, L], BF, name=f"yT{p}", tag=f"yT{p}") for p in range(RT)]

    def emit_attention(p, ic):
        kt, qt = kT[p], qT[p]
        o0 = ps_pv.tile([VW, ICS], F32, name=f"o0_{p}_{ic}", tag="ps_pv")
        o1 = ps_pv.tile([VW, ICS], F32, name=f"o1_{p}_{ic}", tag="ps_pv")
        for j in range(NJ):
            st_ = ps_st.tile([128, 2 * ICS], F32, name=f"st{p}_{ic}_{j}", tag="ps_st")
            nc.tensor.matmul(st_[:, 0:ICS], kt[0:64, ts(j, 128)],
                             qt[0:64, ts(ic, ICS)], start=True, stop=True)
            nc.tensor.matmul(st_[:, ICS:2 * ICS], kt[64:128, ts(j, 128)],
                             qt[64:128, ts(ic, ICS)], start=True, stop=True)
            pt = ptp.tile([128, 2 * ICS], BF, name=f"pt{p}_{ic}_{j}", tag="pt")
            nc.scalar.activation(pt[:], st_[:], EXP, scale=0.125)
            va3 = v_aug[j].rearrange("p (h w) -> p h w", w=VW)
            nc.tensor.matmul(o0[:], va3[:, 2 * p, :], pt[:, 0:ICS],
                             start=(j == 0), stop=(j == NJ - 1))
            nc.tensor.matmul(o1[:], va3[:, 2 * p + 1, :], pt[:, ICS:2 * ICS],
                             start=(j == 0), stop=(j == NJ - 1))
        # finalize: normalize by ones-column sums, write yT slices
        for hh, o in ((0, o0), (1, o1)):
            recip = dsb.tile([1, ICS], F32, name=f"recip{p}_{ic}_{hh}", tag="recip")
            nc.vector.reciprocal(recip[:], o[D:VW, :])
            dsb_t = dsb.tile([D, ICS], F32, name=f"dsb{p}_{ic}_{hh}", tag="dsb")
            nc.gpsimd.partition_broadcast(dsb_t[:], recip[:])
            nc.vector.tensor_mul(yT[p][ds(64 * hh, 64), ts(ic, ICS)],
                                 o[0:D, :], dsb_t[:])

    # ---------------- out projection for the 4 token tiles of one chunk ----------
    def emit_outproj(ic):
        for lt in range(4 * ic, 4 * ic + 4):
            for eh in range(2):
                ps = ps_mm.tile([128, 512], F32, name=f"op_ps{lt}_{eh}", tag="ps_mm")
                for p in range(RT):
                    nc.tensor.matmul(ps[:], yT[p][:, ts(lt, 128)],
                                     woutT[p][:, ts(eh, 512)],
                                     start=(p == 0), stop=(p == RT - 1))
                ob = outp.tile([128, 512], F32, name=f"ob{lt}_{eh}", tag="ob")
                nc.vector.tensor_copy(ob[:], ps[:])
                nc.sync.dma_start(out_d[ts(lt, 128), ts(eh, 512)], ob[:])

    emit_v_proj(range(0, 8))
    emit_qk_proj(0)
    emit_v_proj(range(8, LT))
    for p in range(RT):
        for ic in range(NIC):
            emit_attention(p, ic)
            if ic == 1 and p + 1 < RT:
                emit_qk_proj(p + 1)
            if p == RT - 1:
                emit_outproj(ic)


def build_nc():
    nc = bacc.Bacc("TRN2", target_bir_lowering=False, debug=False,
                   num_devices=N_CORES)
    ins = {
        "xT": nc.dram_tensor("xT", [E, L], BF, kind="ExternalInput")[:],
        "wcqT": nc.dram_tensor("wcqT", [E, R], BF, kind="ExternalInput")[:],
        "wckT": nc.dram_tensor("wckT", [E, R], BF, kind="ExternalInput")[:],
        "wcvT": nc.dram_tensor("wcvT", [E, R], BF, kind="ExternalInput")[:],
        "woutT": nc.dram_tensor("woutT", [R, E], BF, kind="ExternalInput")[:],
    }
    outs = {"out": nc.dram_tensor("out", [L, E], F32, kind="ExternalOutput")[:]}
    with tile.TileContext(nc) as tc:
        emit(tc, outs, ins)
    nc.compile()
    return nc


def shard_inputs(x, W_q, W_k, W_v, W_qp, W_kp, W_vp, W_out):
    """Host-side shard prep: fold the per-stream projection pair into one
    combined matrix (reassociation), slice per head-group, round to the
    bf16 compute dtype, and lay out operands for the device program."""
    import ml_dtypes
    bf16 = ml_dtypes.bfloat16
    x = np.asarray(x, dtype=np.float32)
    # (x @ Wq.T) @ Wqp.T == x @ (Wqp @ Wq).T ; feature-major lhsT layout wants
    # Wc^T = Wq.T @ Wqp.T  with embed on the partition axis.
    wcqT = np.asarray(W_q, np.float32).T @ np.asarray(W_qp, np.float32).T  # [E, E]
    wckT = np.asarray(W_k, np.float32).T @ np.asarray(W_kp, np.float32).T
    wcvT = np.asarray(W_v, np.float32).T @ np.asarray(W_vp, np.float32).T
    woutT = np.asarray(W_out, np.float32).T                               # [E, E]
    in_maps = []
    for c in range(N_CORES):
        b, g = c // 2, c % 2
        sl = slice(g * R, (g + 1) * R)
        in_maps.append({
            "xT": np.ascontiguousarray(x[b].T).astype(bf16),
            "wcqT": np.ascontiguousarray(wcqT[:, sl]).astype(bf16),
            "wckT": np.ascontiguousarray(wckT[:, sl]).astype(bf16),
            "wcvT": np.ascontiguousarray(wcvT[:, sl]).astype(bf16),
            "woutT": np.ascontiguousarray(woutT[sl, :]).astype(bf16),
        })
    return in_maps


_NC = None


def kernel(**inputs):
    global _NC
    from concourse.bass_utils import run_bass_kernel_spmd
    if _NC is None:
        _NC = build_nc()
    in_maps = shard_inputs(**inputs)
    res = run_bass_kernel_spmd(_NC, in_maps, core_ids=list(range(N_CORES)))
    out = np.zeros((4, L, E), dtype=np.float32)
    for c in range(N_CORES):
        out[c // 2] += res.results[c]["out"]
    return out


# revision 20
# speedup vs baseline: 1.6922x; 1.6922x over previous
"""Multi-head self-attention Trainium2 kernel (8-core SPMD).

Sharding: core c handles (batch b = c // 2, head-group g = c % 2); the two
head-group partials per batch are summed on the host (tensor-parallel
unshard, as in the sharding hint's "all-reduce after out_proj").

Host-side shard prep folds the two projection matrices per stream
(W_qp @ W_q etc.) into one combined matrix — pure reassociation of the
linear chain — and pre-rounds all operands to the bf16 compute dtype.

Per-core device program (feature-major activations):
  qT = Wc_q^T.T-contraction of x^T   [512, 2048]   (same for kT)
  v  = token-major projection with a ones column per head (v_aug)
  per head pair, per 512-query chunk:
    S^T tile [128 keys, 2 x 512] via row-packed K=64 matmul pairs
    P^T = exp(S^T / 8) on ScalarE (PSUM -> SBUF bf16)
    O^T[65, 512] += v_aug_h.T @ P^T  (row 64 accumulates softmax denoms)
    normalize via reciprocal + ones-outer-product broadcast, into yT bf16
  out = yT.T-contraction with W_out^T slice, accumulated over pairs.
"""

import numpy as np
from contextlib import ExitStack

import concourse.bass as bass
import concourse.tile as tile
from concourse import mybir, bacc
from concourse.bass import ts, ds

FP = mybir.dt.float32r
BF = mybir.dt.bfloat16
F32 = mybir.dt.float32
EXP = mybir.ActivationFunctionType.Exp

N_CORES = 8
E = 1024          # embed dim
R = 512           # per-core head-group width (8 heads x 64)
L = 2048          # sequence length
D = 64            # head dim
ET = E // 128     # 8 embed tiles
RT = R // 128     # 4 r tiles == head pairs
LT = L // 128     # 16 token tiles
NJ = L // 128     # 16 key tiles
ICS = 512         # query chunk per PV accumulation
NIC = L // ICS    # 4
VW = 65           # v_aug width per head (64 + ones)


def emit(tc, outs, ins, repeat=1):
    with ExitStack() as ctx:
        _emit_body(tc, ctx, outs, ins, repeat)


def _emit_body(tc, ctx, outs, ins, repeat=1):
    nc = tc.nc

    x_d = ins["xT"]          # [1024, 2048] bf16  (x[b].T)
    wc_d = {"wq": ins["wcqT"], "wk": ins["wckT"], "wv": ins["wcvT"]}  # [1024, 512] bf16
    woutT_d = ins["woutT"]   # [512, 1024] bf16
    out_d = outs["out"]      # [2048, 1024] f32

    # ---------------- pools ----------------
    persist = ctx.enter_context(tc.tile_pool(name="persist", bufs=1))  # long-lived sbuf
    ptp = ctx.enter_context(tc.tile_pool(name="ptp", bufs=6))          # exp outputs
    dsb = ctx.enter_context(tc.tile_pool(name="dsb", bufs=2))          # denom broadcast
    outp = ctx.enter_context(tc.tile_pool(name="outp", bufs=2))        # final out staging
    ps_mm = ctx.enter_context(tc.tile_pool(name="ps_mm", space="PSUM", bufs=2))   # [128,512] proj/outproj
    ps_st = ctx.enter_context(tc.tile_pool(name="ps_st", space="PSUM", bufs=2))   # [128,1024] scores
    ps_pv = ctx.enter_context(tc.tile_pool(name="ps_pv", space="PSUM", bufs=2))   # [65,512] pv accum

    def body():
        _emit_once(tc, persist, ptp, dsb, outp, ps_mm, ps_st, ps_pv, outs, ins)

    for it in range(repeat):
        body()
        if it < repeat - 1:
            tc.strict_bb_all_engine_barrier()


def _emit_once(tc, persist, ptp, dsb, outp, ps_mm, ps_st, ps_pv, outs, ins):
    nc = tc.nc
    x_d = ins["xT"]
    wc_d = {"wq": ins["wcqT"], "wk": ins["wckT"], "wv": ins["wcvT"]}
    woutT_d = ins["woutT"]
    out_d = outs["out"]

    # ---------------- warm the Exp table off the critical path ----
    warm = persist.tile([1, 8], BF, name="warm", tag="warm")
    nc.vector.memset(warm[:], 0.0)
    warm2 = persist.tile([1, 8], BF, name="warm2", tag="warm2")
    nc.scalar.activation(warm2[:], warm[:], EXP, scale=0.125)

    # ---------------- load combined weights + x^T (DMA order: qk first) -------
    def load_wc(wname):
        w3 = wc_d[wname].rearrange("(t p) r -> t p r", p=128)  # [8,128,512]
        tiles = []
        for e in range(ET):
            wb = persist.tile([128, R], BF, name=f"{wname}c{e}", tag=f"{wname}c{e}")
            nc.sync.dma_start(wb[:], w3[e])
            tiles.append(wb)
        return tiles

    x3 = x_d.rearrange("(t p) l -> t p l", p=128)  # [8,128,2048]
    xT = [persist.tile([128, L], BF, name=f"xT{t}", tag=f"xT{t}") for t in range(ET)]

    def load_x_chunk(lc):
        for t in range(ET):
            nc.sync.dma_start(xT[t][:, ts(lc, 512)], x3[t, :, ts(lc, 512)])

    wc = {}
    wc["wq"] = load_wc("wq")
    wc["wk"] = load_wc("wk")
    load_x_chunk(0)
    wc["wv"] = load_wc("wv")
    for lc in range(1, 4):
        load_x_chunk(lc)

    wo3 = woutT_d.rearrange("(t p) e -> t p e", p=128)  # [4,128,1024]
    woutT = []
    for t in range(RT):
        wb = persist.tile([128, E], BF, name=f"woutT{t}", tag=f"woutT{t}")
        nc.sync.dma_start(wb[:], wo3[t])
        woutT.append(wb)

    # ---------------- v projection (token-major, with ones cols) ----------------
    v_aug = [persist.tile([128, 8 * VW], BF, name=f"vaug{lt}", tag=f"vaug{lt}")
             for lt in range(LT)]

    def emit_v_proj(lts):
        for lt in lts:
            va = v_aug[lt]
            va3 = va.rearrange("p (h w) -> p h w", w=VW)
            nc.vector.memset(va[:], 1.0)
            ps = ps_mm.tile([128, R], F32, name=f"v_ps{lt}", tag="ps_mm")
            for e in range(ET):
                nc.tensor.matmul(ps[:], xT[e][:, ts(lt, 128)], wc["wv"][e][:],
                                 start=(e == 0), stop=(e == ET - 1))
            ps3 = ps.rearrange("p (h d) -> p h d", d=D)
            nc.vector.tensor_copy(va3[:, :, 0:D], ps3[:])

    # ---------------- q/k projections for one pair ----------------
    qT = [None] * RT
    kT = [None] * RT

    def emit_qk_proj(p):
        for which, store in (("wq", qT), ("wk", kT)):
            dst = persist.tile([128, L], BF, name=f"{which}T{p}", tag=f"{which}T{p}")
            for lc in range(4):
                ps = ps_mm.tile([128, 512], F32, name=f"qk_ps{p}_{which}_{lc}", tag="ps_mm")
                for e in range(ET):
                    nc.tensor.matmul(ps[:], wc[which][e][:, ts(p, 128)],
                                     xT[e][:, ts(lc, 512)],
                                     start=(e == 0), stop=(e == ET - 1))
                nc.vector.tensor_copy(dst[:, ts(lc, 512)], ps[:])
            store[p] = dst

    # ---------------- attention (one pair, one query chunk) ----------------
    yT = [persist.tile([128, L], BF, name=f"yT{p}", tag=f"yT{p}") for p in range(RT)]

    def emit_attention(p, ic):
        kt, qt = kT[p], qT[p]
        o0 = ps_pv.tile([VW, ICS], F32, name=f"o0_{p}_{ic}", tag="ps_pv")
        o1 = ps_pv.tile([VW, ICS], F32, name=f"o1_{p}_{ic}", tag="ps_pv")
        for j in range(NJ):
            st_ = ps_st.tile([128, 2 * ICS], F32, name=f"st{p}_{ic}_{j}", tag="ps_st")
            nc.tensor.matmul(st_[:, 0:ICS], kt[0:64, ts(j, 128)],
                             qt[0:64, ts(ic, ICS)], start=True, stop=True)
            nc.tensor.matmul(st_[:, ICS:2 * ICS], kt[64:128, ts(j, 128)],
                             qt[64:128, ts(ic, ICS)], start=True, stop=True)
            pt = ptp.tile([128, 2 * ICS], BF, name=f"pt{p}_{ic}_{j}", tag="pt")
            nc.scalar.activation(pt[:], st_[:], EXP, scale=0.125)
            va3 = v_aug[j].rearrange("p (h w) -> p h w", w=VW)
            nc.tensor.matmul(o0[:], va3[:, 2 * p, :], pt[:, 0:ICS],
                             start=(j == 0), stop=(j == NJ - 1))
            nc.tensor.matmul(o1[:], va3[:, 2 * p + 1, :], pt[:, ICS:2 * ICS],
                             start=(j == 0), stop=(j == NJ - 1))
        # finalize: normalize by ones-column sums, write yT slices
        for hh, o in ((0, o0), (1, o1)):
            recip = dsb.tile([1, ICS], F32, name=f"recip{p}_{ic}_{hh}", tag="recip")
            nc.vector.reciprocal(recip[:], o[D:VW, :])
            dsb_t = dsb.tile([D, ICS], F32, name=f"dsb{p}_{ic}_{hh}", tag="dsb")
            nc.gpsimd.partition_broadcast(dsb_t[:], recip[:])
            nc.vector.tensor_mul(yT[p][ds(64 * hh, 64), ts(ic, ICS)],
                                 o[0:D, :], dsb_t[:])

    # ---------------- out projection for the 4 token tiles of one chunk ----------
    def emit_outproj(ic):
        for lt in range(4 * ic, 4 * ic + 4):
            for eh in range(2):
                ps = ps_mm.tile([128, 512], F32, name=f"op_ps{lt}_{eh}", tag="ps_mm")
                for p in range(RT):
                    nc.tensor.matmul(ps[:], yT[p][:, ts(lt, 128)],
                                     woutT[p][:, ts(eh, 512)],
                                     start=(p == 0), stop=(p == RT - 1))
                ob = outp.tile([128, 512], F32, name=f"ob{lt}_{eh}", tag="ob")
                nc.vector.tensor_copy(ob[:], ps[:])
                nc.sync.dma_start(out_d[ts(lt, 128), ts(eh, 512)], ob[:])

    emit_v_proj(range(0, 8))
    emit_qk_proj(0)
    emit_v_proj(range(8, LT))
    for p in range(RT):
        for ic in range(NIC):
            emit_attention(p, ic)
            if ic == 1 and p + 1 < RT:
                emit_qk_proj(p + 1)
            if p == RT - 1:
                emit_outproj(ic)


def build_nc(repeat=1):
    nc = bacc.Bacc("TRN2", target_bir_lowering=False, debug=False,
                   num_devices=N_CORES)
    ins = {
        "xT": nc.dram_tensor("xT", [E, L], BF, kind="ExternalInput")[:],
        "wcqT": nc.dram_tensor("wcqT", [E, R], BF, kind="ExternalInput")[:],
        "wckT": nc.dram_tensor("wckT", [E, R], BF, kind="ExternalInput")[:],
        "wcvT": nc.dram_tensor("wcvT", [E, R], BF, kind="ExternalInput")[:],
        "woutT": nc.dram_tensor("woutT", [R, E], BF, kind="ExternalInput")[:],
    }
    outs = {"out": nc.dram_tensor("out", [L, E], F32, kind="ExternalOutput")[:]}
    with tile.TileContext(nc) as tc:
        emit(tc, outs, ins, repeat=repeat)
    nc.compile()
    return nc


def shard_inputs(x, W_q, W_k, W_v, W_qp, W_kp, W_vp, W_out):
    """Host-side shard prep: fold the per-stream projection pair into one
    combined matrix (reassociation), slice per head-group, round to the
    bf16 compute dtype, and lay out operands for the device program."""
    import ml_dtypes
    bf16 = ml_dtypes.bfloat16
    x = np.asarray(x, dtype=np.float32)
    # (x @ Wq.T) @ Wqp.T == x @ (Wqp @ Wq).T ; feature-major lhsT layout wants
    # Wc^T = Wq.T @ Wqp.T  with embed on the partition axis.
    wcqT = np.asarray(W_q, np.float32).T @ np.asarray(W_qp, np.float32).T  # [E, E]
    wckT = np.asarray(W_k, np.float32).T @ np.asarray(W_kp, np.float32).T
    wcvT = np.asarray(W_v, np.float32).T @ np.asarray(W_vp, np.float32).T
    woutT = np.asarray(W_out, np.float32).T                               # [E, E]
    in_maps = []
    for c in range(N_CORES):
        b, g = c // 2, c % 2
        sl = slice(g * R, (g + 1) * R)
        in_maps.append({
            "xT": np.ascontiguousarray(x[b].T).astype(bf16),
            "wcqT": np.ascontiguousarray(wcqT[:, sl]).astype(bf16),
            "wckT": np.ascontiguousarray(wckT[:, sl]).astype(bf16),
            "wcvT": np.ascontiguousarray(wcvT[:, sl]).astype(bf16),
            "woutT": np.ascontiguousarray(woutT[sl, :]).astype(bf16),
        })
    return in_maps


_NC = None


def kernel(**inputs):
    global _NC
    from concourse.bass_utils import run_bass_kernel_spmd
    if _NC is None:
        _NC = build_nc()
    in_maps = shard_inputs(**inputs)
    res = run_bass_kernel_spmd(_NC, in_maps, core_ids=list(range(N_CORES)))
    out = np.zeros((4, L, E), dtype=np.float32)
    for c in range(N_CORES):
        out[c // 2] += res.results[c]["out"]
    return out
